# revision 14
# baseline (speedup 1.0000x reference)
"""Trainium2 Bass kernel: full 1-D convolution (2,097,152 samples x 32,000-tap
RIR) + peak-normalization on 8 NeuronCores.

v6 = fp8 DoubleRow + odd-even Karatsuba L1, with a SECOND odd-even level on
BOTH the B and S sub-convolutions (A stays at L1):

  L1: A = Me*xe, B = Mo*xo, S = Msum*xs  (xs = xe+xo, Msum = Me+Mo)
      y[2m] = A[m] + B[m-1];  y[2m+1] = S[m] - A[m] - B[m]
  L2 on X in {B, S}: quarter-phase split of operand and weights:
      X[2n] = A2x[n] + B2x[n-1];  X[2n+1] = S2x[n] - A2x[n] - B2x[n]

PE column-units ~104us/core.  PSUM = 8 banks exactly: pA x2 (double) + two
single-buffered trios (their sweeps follow the A sweep, long after the
previous combine drained those banks); warm-up matmuls reuse pA0.  Weights
ride the sync queue in consumption order with sim-tuned batch boundaries
(wa 2,2,4,10,16,29; each trio a single 32-pair batch) -- DMA batch-boundary
semaphores resonate with sweep consumption, worth ~6us.  Timeline:
126,011 ns (baseline 231,978; fp8 179,136; Kara-L1 139,715; S-trio 132,952;
+B-trio 131,872); rel_max vs the f64 oracle 2.6e-3 (threshold 2e-2).
"""
import numpy as np

B = 128                      # block size / partition count
KLEN = 32_000                # RIR taps
N = 2_097_152                # signal samples
NOUT = N + KLEN - 1          # full-convolution output length
NSUB = 126                   # chunks per L1 sub-conv (parity split of 252)
G = NSUB // 2                # 63 DoubleRow chunk pairs per L1 sub-conv
GT = 32                      # trio DR pairs (63 quarter-chunks + 1 zero pad)
NCORES = 8
BLK_PER_CORE = 2080          # output blocks per core
HB_PER_CORE = 1040           # output half-blocks per core
QB_PER_CORE = 520            # output quarter-blocks per core
XW = 1170                    # x operand width per core (cols, even)
XPAD = 128                   # window lo = XPAD + base - 2g stays >= 4, even
GROUPS = [(0, 416), (416, 416), (832, 208)]   # (half-col base, Fw)
SX = 16.0                    # x pre-quant scale
SH = 1024.0                  # h pre-quant scale

_NC_CACHE = None


def _build_nc(collective=True, passes=1, dbg_trio=True, dbg_l1=True):
    import concourse.bacc as bacc
    import concourse.bass as bass
    import concourse.mybir as mybir
    from concourse import tile

    f32 = mybir.dt.float32
    f8 = mybir.dt.float8e4
    DR = mybir.MatmulPerfMode.DoubleRow
    add = mybir.AluOpType.add
    sub = mybir.AluOpType.subtract

    nc = bacc.Bacc("TRN2", target_bir_lowering=False, debug=False,
                   num_devices=NCORES)

    # x ops: 0=xe 1=xo0 2=xo1 3=xos 4=xs0 5=xs1 6=xss
    x_in = nc.dram_tensor("x", [B, 7, 2, 2, XW], f8, kind="ExternalInput")
    # L1 A weights (Me): (pair g, stream, ktile, r)
    wa_in = nc.dram_tensor("wa", [B, G, 2, 2, B], f8, kind="ExternalInput")
    # B-trio weights: (pair e, sub 0=Mo0 1=Mo1 2=Mosum, stream, ktile, r)
    wtb_in = nc.dram_tensor("wtb", [B, GT, 3, 2, 2, B], f8,
                            kind="ExternalInput")
    # S-trio weights: (pair e, sub 0=Ms0 1=Ms1 2=Mssum, stream, ktile, r)
    wts_in = nc.dram_tensor("wts", [B, GT, 3, 2, 2, B], f8,
                            kind="ExternalInput")
    y_out = nc.dram_tensor("y", [B, QB_PER_CORE, 2, 2], f32,
                           kind="ExternalOutput")

    with tile.TileContext(nc) as tc:
        with (
            tc.tile_pool(name="data", bufs=1) as data_pool,
            tc.tile_pool(name="wpool", bufs=1) as wpool,
            tc.tile_pool(name="ps", bufs=1, space="PSUM") as ps_pool,
            tc.tile_pool(name="dram", bufs=1, space="DRAM") as dram_pool,
        ):
            x_sb = data_pool.tile([B, 7, 2, 2, XW], f8, name="x_sb")
            # y layout (n, p, e): block 4n + 2p + e  (flat == block-major)
            y_sb = data_pool.tile([B, QB_PER_CORE, 2, 2], f32, name="y_sb")
            mx = data_pool.tile([B, 4], f32, name="mx")
            wa_sb = wpool.tile([B, G, 2, 2, B], f8, name="wa_sb")
            wtb_sb = wpool.tile([B, GT, 3, 2, 2, B], f8, name="wtb_sb")
            wts_sb = wpool.tile([B, GT, 3, 2, 2, B], f8, name="wts_sb")
            a_sb = data_pool.tile([B, 416], f32, name="a_sb")
            ab_sb = data_pool.tile([B, 212], f32, name="ab_sb")
            q1_sb = data_pool.tile([B, 212], f32, name="q1_sb")
            tbx_sb = data_pool.tile([B, 212], f32, name="tbx_sb")
            tb0_sb = data_pool.tile([B, 208], f32, name="tb0_sb")
            t1e_sb = data_pool.tile([B, 208], f32, name="t1e_sb")
            t1o_sb = data_pool.tile([B, 208], f32, name="t1o_sb")
            s_sb = data_pool.tile([B, 208], f32, name="s_sb")
            u_sb = data_pool.tile([B, 208], f32, name="u_sb")
            w2_sb = data_pool.tile([B, 208], f32, name="w2_sb")
            v_sb = data_pool.tile([B, 208], f32, name="v_sb")

            # psum tiles (8 banks exactly; warm-up reuses pA0)
            pAs = [ps_pool.tile([B, 416], f32, name=f"pA{k}", tag=f"pA{k}")
                   for k in range(2)]
            pAb = ps_pool.tile([B, 212], f32, name="pAb", tag="pAb")
            pBb = ps_pool.tile([B, 212], f32, name="pBb", tag="pBb")
            pSb = ps_pool.tile([B, 212], f32, name="pSb", tag="pSb")
            pSA = ps_pool.tile([B, 208], f32, name="pSA", tag="pSA")
            pSB = ps_pool.tile([B, 212], f32, name="pSB", tag="pSB")
            pSS = ps_pool.tile([B, 208], f32, name="pSS", tag="pSS")

            # warm the PE (p-state ramp) while the first x operand lands
            warm = data_pool.tile([B, 512], mybir.dt.bfloat16, name="warm")
            nc.gpsimd.memset(warm[:], 0.0)
            for _ in range(8):
                nc.tensor.matmul(pAs[0][:, :256], warm[:, :B], warm[:, :256],
                                 start=True, stop=True)

            # per-operand x DMAs on the ACT queue (A sweep unblocks ~1.7us in)
            for opi in range(7):
                nc.scalar.dma_start(x_sb[:, opi], x_in[:, opi])
            # weights in consumption order on the sync queue: Me section
            # (small lead batches), Mo section, trio d-pair-major.
            def wbatches(total, lead):
                ws = list(lead)
                while sum(ws) < total:
                    ws.append(min(16, total - sum(ws)))
                return ws
            b0 = 0
            for wn in wbatches(G, (2, 2, 4, 10, 16, 29)):
                nc.sync.dma_start(wa_sb[:, b0:b0 + wn], wa_in[:, b0:b0 + wn])
                b0 += wn
            b0 = 0
            for wn in wbatches(GT, (32,)):
                nc.sync.dma_start(wtb_sb[:, b0:b0 + wn],
                                  wtb_in[:, b0:b0 + wn])
                b0 += wn
            b0 = 0
            for wn in wbatches(GT, (32,)):
                nc.sync.dma_start(wts_sb[:, b0:b0 + wn],
                                  wts_in[:, b0:b0 + wn])
                b0 += wn

            def one_pass():
                for gi, (base, Fw) in enumerate(GROUPS):
                    Fh = Fw // 2           # quarter-cols in this group
                    qb = base // 2         # quarter base
                    pA = pAs[gi % 2]

                    # L1 A sweep (half-block windows)
                    out = pA[:, :Fw]
                    for g in range(G):
                        lo = XPAD + base - 2 * g
                        rhs_hi = x_sb[:, 0, 0, :, lo:lo + Fw]
                        rhs_lo = x_sb[:, 0, 1, :, lo:lo + Fw]
                        nc.tensor.matmul(out, wa_sb[:, g, 0], rhs_hi,
                                         start=(g == 0), stop=False,
                                         perf_mode=DR)
                        nc.tensor.matmul(out, wa_sb[:, g, 1], rhs_hi,
                                         start=False, stop=False,
                                         perf_mode=DR)
                        nc.tensor.matmul(out, wa_sb[:, g, 0], rhs_lo,
                                         start=False, stop=(g == G - 1),
                                         perf_mode=DR)

                    # B-trio sweep (quarter windows; psums shifted: pAb/pSb
                    # by 1, pBb by 2, widths Fh+4/Fh+2 with even guards)
                    for e in range(GT):
                        lo = XPAD + qb - 2 * e
                        for st, ps, Fi in ((1, pAb, Fh + 2), (2, pBb, Fh + 4),
                                           (3, pSb, Fh + 2)):
                            rhs_hi = x_sb[:, st, 0, :, lo:lo + Fi]
                            rhs_lo = x_sb[:, st, 1, :, lo:lo + Fi]
                            wsl = wtb_sb[:, e, st - 1]
                            out = ps[:, :Fi]
                            nc.tensor.matmul(out, wsl[:, 0], rhs_hi,
                                             start=(e == 0), stop=False,
                                             perf_mode=DR)
                            nc.tensor.matmul(out, wsl[:, 1], rhs_hi,
                                             start=False, stop=False,
                                             perf_mode=DR)
                            nc.tensor.matmul(out, wsl[:, 0], rhs_lo,
                                             start=False, stop=(e == GT - 1),
                                             perf_mode=DR)

                    # trio sweep (quarter-block windows), runs last: the
                    # single-set trio banks are long drained by now
                    for e in range(GT if dbg_trio else 0):
                        lo = XPAD + qb - 2 * e
                        for st, ps, Fi in ((4, pSA, Fh), (5, pSB, Fh + 2),
                                           (6, pSS, Fh)):
                            rhs_hi = x_sb[:, st, 0, :, lo:lo + Fi]
                            rhs_lo = x_sb[:, st, 1, :, lo:lo + Fi]
                            wsl = wts_sb[:, e, st - 4]
                            out = ps[:, :Fi]
                            nc.tensor.matmul(out, wsl[:, 0], rhs_hi,
                                             start=(e == 0), stop=False,
                                             perf_mode=DR)
                            nc.tensor.matmul(out, wsl[:, 1], rhs_hi,
                                             start=False, stop=False,
                                             perf_mode=DR)
                            nc.tensor.matmul(out, wsl[:, 0], rhs_lo,
                                             start=False, stop=(e == GT - 1),
                                             perf_mode=DR)

                    # combine (DVE, all 1-D strided APs):
                    #   pA[n,p] = A[2n+p]; pB[n,p] = B[2n+p-1]
                    #   pSA[n] = A2s[n]; pSB[n] = B2s[n-1]; pSS[n] = S2s[n]
                    # stage pA / pAb in SBUF (stepped PSUM reads crash HW;
                    # the rest are plain/offset PSUM reads, one per op)
                    nc.vector.tensor_copy(a_sb[:, :Fw], pA[:, :Fw])
                    nc.vector.tensor_copy(ab_sb[:, :Fh + 2], pAb[:, :Fh + 2])
                    # q1[c] = (A2b+B2b)[qb-1+c];  tbx[c] = B[2qb+2c-1]
                    nc.vector.scalar_tensor_tensor(
                        q1_sb[:, :Fh + 2], ab_sb[:, :Fh + 2], 0.0,
                        pBb[:, 1:Fh + 3], op0=add, op1=add)
                    nc.vector.scalar_tensor_tensor(
                        tbx_sb[:, :Fh + 2], q1_sb[:, :Fh + 2], -1.0,
                        pSb[:, :Fh + 2], op0=mybir.AluOpType.mult, op1=add)
                    # tb0[c] = B[2(qb+c)]
                    nc.vector.scalar_tensor_tensor(
                        tb0_sb[:, :Fh], ab_sb[:, 1:Fh + 1], 0.0,
                        pBb[:, 1:Fh + 1], op0=add, op1=add)
                    # ye: y[2m] = A[m] + B[m-1]
                    nc.vector.scalar_tensor_tensor(
                        y_sb[:, qb:qb + Fh, 0, 0], a_sb[:, 0:Fw:2], 0.0,
                        tbx_sb[:, :Fh], op0=add, op1=add)
                    nc.vector.scalar_tensor_tensor(
                        y_sb[:, qb:qb + Fh, 1, 0], a_sb[:, 1:Fw:2], 0.0,
                        tb0_sb[:, :Fh], op0=add, op1=add)
                    # t1[m] = A[m] + B[m]
                    nc.vector.scalar_tensor_tensor(
                        t1e_sb[:, :Fh], a_sb[:, 0:Fw:2], 0.0,
                        tb0_sb[:, :Fh], op0=add, op1=add)
                    nc.vector.scalar_tensor_tensor(
                        t1o_sb[:, :Fh], a_sb[:, 1:Fw:2], 0.0,
                        tbx_sb[:, 1:Fh + 1], op0=add, op1=add)
                    # S[2n] = pSA + pSB ; S[2n+1] = pSS - pSA - pSB[+1]
                    nc.vector.tensor_copy(s_sb[:, :Fh], pSA[:, :Fh])
                    nc.vector.scalar_tensor_tensor(
                        u_sb[:, :Fh], s_sb[:, :Fh], 0.0, pSB[:, :Fh],
                        op0=add, op1=add)
                    nc.vector.scalar_tensor_tensor(
                        w2_sb[:, :Fh], s_sb[:, :Fh], 0.0, pSB[:, 1:Fh + 1],
                        op0=add, op1=add)
                    nc.vector.scalar_tensor_tensor(
                        v_sb[:, :Fh], w2_sb[:, :Fh], -1.0, pSS[:, :Fh],
                        op0=mybir.AluOpType.mult, op1=add)  # pSS - w2
                    # yo: y[2m+1] = S[m] - t1[m]
                    nc.vector.scalar_tensor_tensor(
                        y_sb[:, qb:qb + Fh, 0, 1], u_sb[:, :Fh], 0.0,
                        t1e_sb[:, :Fh], op0=add, op1=sub)
                    nc.vector.scalar_tensor_tensor(
                        y_sb[:, qb:qb + Fh, 1, 1], v_sb[:, :Fh], 0.0,
                        t1o_sb[:, :Fh], op0=add, op1=sub)
                    nc.vector.tensor_reduce(
                        mx[:, gi:gi + 1], y_sb[:, qb:qb + Fh, :, :],
                        axis=mybir.AxisListType.XYZ,
                        op=mybir.AluOpType.max, apply_absolute_value=True,
                    )

                # local scalar max -> all partitions
                am = data_pool.tile([B, 1], f32, name="am")
                nc.vector.tensor_reduce(
                    am[:], mx[:, :3], axis=mybir.AxisListType.X,
                    op=mybir.AluOpType.max,
                )
                gm = data_pool.tile([B, 1], f32, name="gm")
                nc.gpsimd.partition_all_reduce(
                    gm[:], am[:], B, bass.bass_isa.ReduceOp.max
                )

                scb = data_pool.tile([B, 1], f32, name="scb")
                if collective:
                    cc_in = dram_pool.tile([B, 1], f32, name="cc_in")
                    cc_out = dram_pool.tile([B, 1], f32, name="cc_out",
                                            addr_space="Shared")
                    nc.sync.dma_start(cc_in[:], gm[:])
                    nc.gpsimd.collective_compute(
                        "AllReduce",
                        mybir.AluOpType.max,
                        replica_groups=[list(range(NCORES))],
                        ins=[cc_in[:].opt()],
                        outs=[cc_out[:].opt()],
                    )
                    nc.sync.dma_start(scb[:], cc_out[:])
                else:
                    nc.vector.tensor_copy(scb[:], gm[:])

                # y_sb holds y*SX*SH; y_out = y_sb/max(gm, SX*SH)
                nc.vector.tensor_scalar_max(scb[:], scb[:], SX * SH)
                nc.vector.reciprocal(scb[:], scb[:])
                for t in range(5):
                    sl = slice(t * 104, (t + 1) * 104)
                    nc.vector.tensor_scalar_mul(y_sb[:, sl, :, :],
                                                y_sb[:, sl, :, :],
                                                scb[:, 0:1])
                    eng = nc.sync if t % 2 == 0 else nc.scalar
                    eng.dma_start(y_out[:, sl, :, :], y_sb[:, sl, :, :])

            for _ in range(passes):
                one_pass()

    nc.compile()
    return nc


def _q8(v):
    import ml_dtypes
    return np.clip(np.asarray(v, np.float32), -240.0, 240.0).astype(
        ml_dtypes.float8_e4m3)


EXTLEN = 33_300


def _toep(ext, stride, shift, nchunk):
    """W[c, s, r] = ext[stride*c + (r - s) + shift]  -> [s, nchunk, r]."""
    idx = shift + np.arange(B)[None, :] - np.arange(B)[:, None]
    offs = stride * np.arange(nchunk)
    W = ext[offs[:, None, None] + idx[None, :, :]]
    return W.transpose(1, 0, 2)


def _pack_pairs(Wsr, npair):
    """[s, nchunk, r] -> [s, npair, 2, r], zero-padding odd chunk counts."""
    s, nch, r = Wsr.shape
    out = np.zeros((s, npair * 2, r), np.float32)
    out[:, :nch] = Wsr
    return out.reshape(s, npair, 2, r)


def _build_weights(h):
    """Returns (wa [B,G,2,2,B], wtb, wts [B,GT,3,2,2,B]) fp8."""
    import ml_dtypes
    ext = np.zeros(EXTLEN, np.float32)
    ext[B:B + KLEN] = h                    # ext[v] = h[v-128]
    ext2 = ext + np.concatenate([ext[256:], np.zeros(256, np.float32)])
    hsum_ext = ext + np.concatenate([ext[B:], np.zeros(B, np.float32)])
    sum2_ext = hsum_ext + np.concatenate(
        [hsum_ext[256:], np.zeros(256, np.float32)])

    def hilo(taps):
        th = _q8(taps * SH)
        tl = _q8(taps * SH - th.astype(np.float32))
        return th.astype(np.float32), tl.astype(np.float32)

    e_h, e_l = hilo(ext)
    e2_h, e2_l = hilo(ext2)
    hs_h, hs_l = hilo(hsum_ext)
    s2_h, s2_l = hilo(sum2_ext)

    wa = np.empty((B, G, 2, 2, B), dtype=ml_dtypes.float8_e4m3)
    for st, t in enumerate((e_h, e_l)):
        wa[:, :, st] = _pack_pairs(
            _toep(t, 256, 128, NSUB), G).astype(ml_dtypes.float8_e4m3)

    def trio(specs):
        wt = np.empty((B, GT, 3, 2, 2, B), dtype=ml_dtypes.float8_e4m3)
        for sb, (th, tl, shift) in enumerate(specs):
            for st, t in enumerate((th, tl)):
                wt[:, :, sb, st] = _pack_pairs(
                    _toep(t, 512, shift, 63), GT).astype(
                    ml_dtypes.float8_e4m3)
        return wt

    wtb = trio(((e_h, e_l, 256), (e_h, e_l, 512), (e2_h, e2_l, 256)))
    wts = trio(((hs_h, hs_l, 128), (hs_h, hs_l, 384), (s2_h, s2_l, 128)))
    return wa, wtb, wts


def _build_x_ops(data):
    """Global col matrices for ops (xe, xo, xs0, xs1, xss) x (hi, lo):
    element k at col PADL + k.  Returns (list[list[np]], PADL)."""
    import ml_dtypes
    NB = N // B
    blocks = data.reshape(NB, B)
    KH = NCORES * HB_PER_CORE
    xe = np.zeros((KH, B), np.float32)
    xo = np.zeros((KH, B), np.float32)
    xe[:NB // 2] = blocks[0::2]
    xo[:NB // 2] = blocks[1::2]
    xs = xe + xo
    xo0 = xo[0::2]
    xo1 = xo[1::2]
    xos = xo0 + xo1
    xs0 = xs[0::2]
    xs1 = xs[1::2]
    xss = xs0 + xs1
    PADL = 132
    out = []
    for op in (xe, xo0, xo1, xos, xs0, xs1, xss):
        hi = _q8(op * SX)
        lo = _q8(op * SX - hi.astype(np.float32))
        mats = []
        for strm in (hi, lo):
            g = np.zeros((B, PADL + KH + 8), dtype=ml_dtypes.float8_e4m3)
            g[:, PADL:PADL + len(strm)] = strm.T
            mats.append(g)
        out.append(mats)
    return out, PADL


def _shard(xops, PADL, i):
    """Core i input: [B, 7, 2, 2, XW] fp8."""
    import ml_dtypes
    K0h = i * HB_PER_CORE
    K0q = i * QB_PER_CORE
    x = np.empty((B, 7, 2, 2, XW), dtype=ml_dtypes.float8_e4m3)
    # (op index, base element index, extra left shift)
    specs = ((0, K0h, 128), (1, K0q, 129), (2, K0q, 130), (3, K0q, 129),
             (4, K0q, 128), (5, K0q, 129), (6, K0q, 128))
    for op, k0, shift in specs:
        base = PADL + k0 - shift
        for st in range(2):
            g = xops[op][st]
            x[:, op, st, 0, :] = g[:, base:base + XW]
            x[:, op, st, 1, :] = g[:, base - 1:base - 1 + XW]
    return x


def kernel(data, rir):
    global _NC_CACHE
    from concourse.bass_utils import run_bass_kernel_spmd

    data = np.asarray(data, dtype=np.float32).reshape(-1)
    h = np.asarray(rir, dtype=np.float32).reshape(-1)

    if _NC_CACHE is None:
        _NC_CACHE = _build_nc()
    nc = _NC_CACHE

    wa, wtb, wts = _build_weights(h)
    xops, PADL = _build_x_ops(data)
    in_maps = [{"x": _shard(xops, PADL, i), "wa": wa, "wtb": wtb,
                "wts": wts} for i in range(NCORES)]
    res = run_bass_kernel_spmd(nc, in_maps, core_ids=list(range(NCORES)))

    y = np.empty(NCORES * BLK_PER_CORE * B, np.float32)
    span = BLK_PER_CORE * B
    for i in range(NCORES):
        y[i * span:(i + 1) * span] = \
            res.results[i]["y"].reshape(B, BLK_PER_CORE).T.reshape(-1)
    return y[:NOUT]


# revision 15
# speedup vs baseline: 1.0124x; 1.0124x over previous
"""Trainium2 Bass kernel: full 1-D convolution (2,097,152 samples x 32,000-tap
RIR) + peak-normalization on 8 NeuronCores.

v6 = fp8 DoubleRow + odd-even Karatsuba L1, with a SECOND odd-even level on
BOTH the B and S sub-convolutions (A stays at L1):

  L1: A = Me*xe, B = Mo*xo, S = Msum*xs  (xs = xe+xo, Msum = Me+Mo)
      y[2m] = A[m] + B[m-1];  y[2m+1] = S[m] - A[m] - B[m]
  L2 on X in {B, S}: quarter-phase split of operand and weights:
      X[2n] = A2x[n] + B2x[n-1];  X[2n+1] = S2x[n] - A2x[n] - B2x[n]

PE column-units ~104us/core.  PSUM = 8 banks exactly: pA x2 (double) + two
single-buffered trios (their sweeps follow the A sweep, long after the
previous combine drained those banks); warm-up matmuls reuse pA0.  Weights
ride the sync queue in consumption order with sim-tuned batch boundaries
(wa 2,2,4,10,16,29; each trio a single 32-pair batch) -- DMA batch-boundary
semaphores resonate with sweep consumption, worth ~6us.  Timeline:
125,844 ns (baseline 231,978; fp8 179,136; Kara-L1 139,715; S-trio 132,952;
+B-trio 131,872); rel_max vs the f64 oracle 2.6e-3 (threshold 2e-2).
"""
import numpy as np

B = 128                      # block size / partition count
KLEN = 32_000                # RIR taps
N = 2_097_152                # signal samples
NOUT = N + KLEN - 1          # full-convolution output length
NSUB = 126                   # chunks per L1 sub-conv (parity split of 252)
G = NSUB // 2                # 63 DoubleRow chunk pairs per L1 sub-conv
GT = 32                      # trio DR pairs (63 quarter-chunks + 1 zero pad)
NCORES = 8
BLK_PER_CORE = 2080          # output blocks per core
HB_PER_CORE = 1040           # output half-blocks per core
QB_PER_CORE = 520            # output quarter-blocks per core
XW = 1170                    # x operand width per core (cols, even)
XPAD = 128                   # window lo = XPAD + base - 2g stays >= 4, even
GROUPS = [(0, 416), (416, 416), (832, 208)]   # (half-col base, Fw)
SX = 16.0                    # x pre-quant scale
SH = 1024.0                  # h pre-quant scale

_NC_CACHE = None


def _build_nc(collective=True, passes=1, dbg_trio=True, dbg_l1=True):
    import concourse.bacc as bacc
    import concourse.bass as bass
    import concourse.mybir as mybir
    from concourse import tile

    f32 = mybir.dt.float32
    f8 = mybir.dt.float8e4
    DR = mybir.MatmulPerfMode.DoubleRow
    add = mybir.AluOpType.add
    sub = mybir.AluOpType.subtract

    nc = bacc.Bacc("TRN2", target_bir_lowering=False, debug=False,
                   num_devices=NCORES)

    # x ops: 0=xe 1=xo0 2=xo1 3=xos 4=xs0 5=xs1 6=xss
    x_in = nc.dram_tensor("x", [B, 7, 2, 2, XW], f8, kind="ExternalInput")
    # L1 A weights (Me): (pair g, stream, ktile, r)
    wa_in = nc.dram_tensor("wa", [B, G, 2, 2, B], f8, kind="ExternalInput")
    # B-trio weights: (pair e, sub 0=Mo0 1=Mo1 2=Mosum, stream, ktile, r)
    wtb_in = nc.dram_tensor("wtb", [B, GT, 3, 2, 2, B], f8,
                            kind="ExternalInput")
    # S-trio weights: (pair e, sub 0=Ms0 1=Ms1 2=Mssum, stream, ktile, r)
    wts_in = nc.dram_tensor("wts", [B, GT, 3, 2, 2, B], f8,
                            kind="ExternalInput")
    y_out = nc.dram_tensor("y", [B, QB_PER_CORE, 2, 2], f32,
                           kind="ExternalOutput")

    with tile.TileContext(nc) as tc:
        with (
            tc.tile_pool(name="data", bufs=1) as data_pool,
            tc.tile_pool(name="wpool", bufs=1) as wpool,
            tc.tile_pool(name="ps", bufs=1, space="PSUM") as ps_pool,
            tc.tile_pool(name="dram", bufs=1, space="DRAM") as dram_pool,
        ):
            x_sb = data_pool.tile([B, 7, 2, 2, XW], f8, name="x_sb")
            # y layout (n, p, e): block 4n + 2p + e  (flat == block-major)
            y_sb = data_pool.tile([B, QB_PER_CORE, 2, 2], f32, name="y_sb")
            mx = data_pool.tile([B, 4], f32, name="mx")
            wa_sb = wpool.tile([B, G, 2, 2, B], f8, name="wa_sb")
            wtb_sb = wpool.tile([B, GT, 3, 2, 2, B], f8, name="wtb_sb")
            wts_sb = wpool.tile([B, GT, 3, 2, 2, B], f8, name="wts_sb")
            a_sb = data_pool.tile([B, 416], f32, name="a_sb")
            ab_sb = data_pool.tile([B, 212], f32, name="ab_sb")
            q1_sb = data_pool.tile([B, 212], f32, name="q1_sb")
            tbx_sb = data_pool.tile([B, 212], f32, name="tbx_sb")
            tb0_sb = data_pool.tile([B, 208], f32, name="tb0_sb")
            t1e_sb = data_pool.tile([B, 208], f32, name="t1e_sb")
            t1o_sb = data_pool.tile([B, 208], f32, name="t1o_sb")
            s_sb = data_pool.tile([B, 208], f32, name="s_sb")
            u_sb = data_pool.tile([B, 208], f32, name="u_sb")
            w2_sb = data_pool.tile([B, 208], f32, name="w2_sb")
            v_sb = data_pool.tile([B, 208], f32, name="v_sb")

            # psum tiles (8 banks exactly; warm-up reuses pA0)
            pAs = [ps_pool.tile([B, 416], f32, name=f"pA{k}", tag=f"pA{k}")
                   for k in range(2)]
            pAb = ps_pool.tile([B, 212], f32, name="pAb", tag="pAb")
            pBb = ps_pool.tile([B, 212], f32, name="pBb", tag="pBb")
            pSb = ps_pool.tile([B, 212], f32, name="pSb", tag="pSb")
            pSA = ps_pool.tile([B, 208], f32, name="pSA", tag="pSA")
            pSB = ps_pool.tile([B, 212], f32, name="pSB", tag="pSB")
            pSS = ps_pool.tile([B, 208], f32, name="pSS", tag="pSS")

            # warm the PE (p-state ramp) while the first x operand lands
            warm = data_pool.tile([B, 512], mybir.dt.bfloat16, name="warm")
            nc.gpsimd.memset(warm[:], 0.0)
            for _ in range(8):
                nc.tensor.matmul(pAs[0][:, :256], warm[:, :B], warm[:, :256],
                                 start=True, stop=True)

            # per-operand x DMAs on the ACT queue (A sweep unblocks ~1.7us in)
            for opi in range(7):
                nc.scalar.dma_start(x_sb[:, opi], x_in[:, opi])
            # weights in consumption order on the sync queue: Me section
            # (small lead batches), Mo section, trio d-pair-major.
            def wbatches(total, lead):
                ws = list(lead)
                while sum(ws) < total:
                    ws.append(min(16, total - sum(ws)))
                return ws
            b0 = 0
            for wn in wbatches(G, (2, 2, 4, 10, 16, 29)):
                nc.sync.dma_start(wa_sb[:, b0:b0 + wn], wa_in[:, b0:b0 + wn])
                b0 += wn
            b0 = 0
            for wn in wbatches(GT, (32,)):
                nc.sync.dma_start(wtb_sb[:, b0:b0 + wn],
                                  wtb_in[:, b0:b0 + wn])
                b0 += wn
            b0 = 0
            for wn in wbatches(GT, (32,)):
                nc.sync.dma_start(wts_sb[:, b0:b0 + wn],
                                  wts_in[:, b0:b0 + wn])
                b0 += wn

            def one_pass():
                for gi, (base, Fw) in enumerate(GROUPS):
                    Fh = Fw // 2           # quarter-cols in this group
                    qb = base // 2         # quarter base
                    pA = pAs[gi % 2]

                    # L1 A sweep (half-block windows)
                    out = pA[:, :Fw]
                    for g in range(G):
                        lo = XPAD + base - 2 * g
                        rhs_hi = x_sb[:, 0, 0, :, lo:lo + Fw]
                        rhs_lo = x_sb[:, 0, 1, :, lo:lo + Fw]
                        nc.tensor.matmul(out, wa_sb[:, g, 0], rhs_hi,
                                         start=(g == 0), stop=False,
                                         perf_mode=DR)
                        nc.tensor.matmul(out, wa_sb[:, g, 1], rhs_hi,
                                         start=False, stop=False,
                                         perf_mode=DR)
                        nc.tensor.matmul(out, wa_sb[:, g, 0], rhs_lo,
                                         start=False, stop=(g == G - 1),
                                         perf_mode=DR)

                    # B-trio sweep (quarter windows; psums shifted: pAb/pSb
                    # by 1, pBb by 2, widths Fh+4/Fh+2 with even guards)
                    for e in range(GT):
                        lo = XPAD + qb - 2 * e
                        for st, ps, Fi in ((1, pAb, Fh + 2), (2, pBb, Fh + 4),
                                           (3, pSb, Fh + 2)):
                            rhs_hi = x_sb[:, st, 0, :, lo:lo + Fi]
                            rhs_lo = x_sb[:, st, 1, :, lo:lo + Fi]
                            wsl = wtb_sb[:, e, st - 1]
                            out = ps[:, :Fi]
                            nc.tensor.matmul(out, wsl[:, 0], rhs_hi,
                                             start=(e == 0), stop=False,
                                             perf_mode=DR)
                            nc.tensor.matmul(out, wsl[:, 1], rhs_hi,
                                             start=False, stop=False,
                                             perf_mode=DR)
                            nc.tensor.matmul(out, wsl[:, 0], rhs_lo,
                                             start=False, stop=(e == GT - 1),
                                             perf_mode=DR)

                    # trio sweep (quarter-block windows), runs last: the
                    # single-set trio banks are long drained by now
                    for e in range(GT if dbg_trio else 0):
                        lo = XPAD + qb - 2 * e
                        for st, ps, Fi in ((4, pSA, Fh), (5, pSB, Fh + 2),
                                           (6, pSS, Fh)):
                            rhs_hi = x_sb[:, st, 0, :, lo:lo + Fi]
                            rhs_lo = x_sb[:, st, 1, :, lo:lo + Fi]
                            wsl = wts_sb[:, e, st - 4]
                            out = ps[:, :Fi]
                            nc.tensor.matmul(out, wsl[:, 0], rhs_hi,
                                             start=(e == 0), stop=False,
                                             perf_mode=DR)
                            nc.tensor.matmul(out, wsl[:, 1], rhs_hi,
                                             start=False, stop=False,
                                             perf_mode=DR)
                            nc.tensor.matmul(out, wsl[:, 0], rhs_lo,
                                             start=False, stop=(e == GT - 1),
                                             perf_mode=DR)

                    # combine (DVE, all 1-D strided APs):
                    #   pA[n,p] = A[2n+p]; pB[n,p] = B[2n+p-1]
                    #   pSA[n] = A2s[n]; pSB[n] = B2s[n-1]; pSS[n] = S2s[n]
                    # stage pA / pAb in SBUF (stepped PSUM reads crash HW;
                    # the rest are plain/offset PSUM reads, one per op)
                    nc.vector.tensor_copy(a_sb[:, :Fw], pA[:, :Fw])
                    nc.vector.tensor_copy(ab_sb[:, :Fh + 2], pAb[:, :Fh + 2])
                    # q1[c] = (A2b+B2b)[qb-1+c];  tbx[c] = B[2qb+2c-1]
                    nc.vector.scalar_tensor_tensor(
                        q1_sb[:, :Fh + 2], ab_sb[:, :Fh + 2], 0.0,
                        pBb[:, 1:Fh + 3], op0=add, op1=add)
                    nc.vector.scalar_tensor_tensor(
                        tbx_sb[:, :Fh + 2], q1_sb[:, :Fh + 2], -1.0,
                        pSb[:, :Fh + 2], op0=mybir.AluOpType.mult, op1=add)
                    # tb0[c] = B[2(qb+c)]
                    nc.vector.scalar_tensor_tensor(
                        tb0_sb[:, :Fh], ab_sb[:, 1:Fh + 1], 0.0,
                        pBb[:, 1:Fh + 1], op0=add, op1=add)
                    # ye: y[2m] = A[m] + B[m-1]
                    nc.vector.scalar_tensor_tensor(
                        y_sb[:, qb:qb + Fh, 0, 0], a_sb[:, 0:Fw:2], 0.0,
                        tbx_sb[:, :Fh], op0=add, op1=add)
                    nc.vector.scalar_tensor_tensor(
                        y_sb[:, qb:qb + Fh, 1, 0], a_sb[:, 1:Fw:2], 0.0,
                        tb0_sb[:, :Fh], op0=add, op1=add)
                    # t1[m] = A[m] + B[m]
                    nc.vector.scalar_tensor_tensor(
                        t1e_sb[:, :Fh], a_sb[:, 0:Fw:2], 0.0,
                        tb0_sb[:, :Fh], op0=add, op1=add)
                    nc.vector.scalar_tensor_tensor(
                        t1o_sb[:, :Fh], a_sb[:, 1:Fw:2], 0.0,
                        tbx_sb[:, 1:Fh + 1], op0=add, op1=add)
                    # S[2n] = pSA + pSB ; S[2n+1] = pSS - pSA - pSB[+1]
                    nc.vector.tensor_copy(s_sb[:, :Fh], pSA[:, :Fh])
                    nc.vector.scalar_tensor_tensor(
                        u_sb[:, :Fh], s_sb[:, :Fh], 0.0, pSB[:, :Fh],
                        op0=add, op1=add)
                    nc.vector.scalar_tensor_tensor(
                        w2_sb[:, :Fh], s_sb[:, :Fh], 0.0, pSB[:, 1:Fh + 1],
                        op0=add, op1=add)
                    nc.vector.scalar_tensor_tensor(
                        v_sb[:, :Fh], w2_sb[:, :Fh], -1.0, pSS[:, :Fh],
                        op0=mybir.AluOpType.mult, op1=add)  # pSS - w2
                    # yo: y[2m+1] = S[m] - t1[m]
                    nc.vector.scalar_tensor_tensor(
                        y_sb[:, qb:qb + Fh, 0, 1], u_sb[:, :Fh], 0.0,
                        t1e_sb[:, :Fh], op0=add, op1=sub)
                    nc.vector.scalar_tensor_tensor(
                        y_sb[:, qb:qb + Fh, 1, 1], v_sb[:, :Fh], 0.0,
                        t1o_sb[:, :Fh], op0=add, op1=sub)
                    nc.vector.tensor_reduce(
                        mx[:, gi:gi + 1], y_sb[:, qb:qb + Fh, :, :],
                        axis=mybir.AxisListType.XYZ,
                        op=mybir.AluOpType.max, apply_absolute_value=True,
                    )

                # local scalar max -> all partitions
                am = data_pool.tile([B, 1], f32, name="am")
                nc.vector.tensor_reduce(
                    am[:], mx[:, :3], axis=mybir.AxisListType.X,
                    op=mybir.AluOpType.max,
                )
                gm = data_pool.tile([B, 1], f32, name="gm")
                nc.gpsimd.partition_all_reduce(
                    gm[:], am[:], B, bass.bass_isa.ReduceOp.max
                )

                scb = data_pool.tile([B, 1], f32, name="scb")
                if collective:
                    cc_in = dram_pool.tile([B, 1], f32, name="cc_in")
                    cc_out = dram_pool.tile([B, 1], f32, name="cc_out",
                                            addr_space="Shared")
                    nc.sync.dma_start(cc_in[:], gm[:])
                    nc.gpsimd.collective_compute(
                        "AllReduce",
                        mybir.AluOpType.max,
                        replica_groups=[list(range(NCORES))],
                        ins=[cc_in[:].opt()],
                        outs=[cc_out[:].opt()],
                    )
                    nc.sync.dma_start(scb[:], cc_out[:])
                else:
                    nc.vector.tensor_copy(scb[:], gm[:])

                # y_sb holds y*SX*SH; y_out = y_sb/max(gm, SX*SH)
                nc.vector.tensor_scalar_max(scb[:], scb[:], SX * SH)
                nc.vector.reciprocal(scb[:], scb[:])
                for t in range(4):
                    sl = slice(t * 130, (t + 1) * 130)
                    nc.vector.tensor_scalar_mul(y_sb[:, sl, :, :],
                                                y_sb[:, sl, :, :],
                                                scb[:, 0:1])
                    eng = nc.sync if t % 2 == 0 else nc.scalar
                    eng.dma_start(y_out[:, sl, :, :], y_sb[:, sl, :, :])

            for _ in range(passes):
                one_pass()

    nc.compile()
    return nc


def _q8(v):
    import ml_dtypes
    return np.clip(np.asarray(v, np.float32), -240.0, 240.0).astype(
        ml_dtypes.float8_e4m3)


EXTLEN = 33_300


def _toep(ext, stride, shift, nchunk):
    """W[c, s, r] = ext[stride*c + (r - s) + shift]  -> [s, nchunk, r]."""
    idx = shift + np.arange(B)[None, :] - np.arange(B)[:, None]
    offs = stride * np.arange(nchunk)
    W = ext[offs[:, None, None] + idx[None, :, :]]
    return W.transpose(1, 0, 2)


def _pack_pairs(Wsr, npair):
    """[s, nchunk, r] -> [s, npair, 2, r], zero-padding odd chunk counts."""
    s, nch, r = Wsr.shape
    out = np.zeros((s, npair * 2, r), np.float32)
    out[:, :nch] = Wsr
    return out.reshape(s, npair, 2, r)


def _build_weights(h):
    """Returns (wa [B,G,2,2,B], wtb, wts [B,GT,3,2,2,B]) fp8."""
    import ml_dtypes
    ext = np.zeros(EXTLEN, np.float32)
    ext[B:B + KLEN] = h                    # ext[v] = h[v-128]
    ext2 = ext + np.concatenate([ext[256:], np.zeros(256, np.float32)])
    hsum_ext = ext + np.concatenate([ext[B:], np.zeros(B, np.float32)])
    sum2_ext = hsum_ext + np.concatenate(
        [hsum_ext[256:], np.zeros(256, np.float32)])

    def hilo(taps):
        th = _q8(taps * SH)
        tl = _q8(taps * SH - th.astype(np.float32))
        return th.astype(np.float32), tl.astype(np.float32)

    e_h, e_l = hilo(ext)
    e2_h, e2_l = hilo(ext2)
    hs_h, hs_l = hilo(hsum_ext)
    s2_h, s2_l = hilo(sum2_ext)

    wa = np.empty((B, G, 2, 2, B), dtype=ml_dtypes.float8_e4m3)
    for st, t in enumerate((e_h, e_l)):
        wa[:, :, st] = _pack_pairs(
            _toep(t, 256, 128, NSUB), G).astype(ml_dtypes.float8_e4m3)

    def trio(specs):
        wt = np.empty((B, GT, 3, 2, 2, B), dtype=ml_dtypes.float8_e4m3)
        for sb, (th, tl, shift) in enumerate(specs):
            for st, t in enumerate((th, tl)):
                wt[:, :, sb, st] = _pack_pairs(
                    _toep(t, 512, shift, 63), GT).astype(
                    ml_dtypes.float8_e4m3)
        return wt

    wtb = trio(((e_h, e_l, 256), (e_h, e_l, 512), (e2_h, e2_l, 256)))
    wts = trio(((hs_h, hs_l, 128), (hs_h, hs_l, 384), (s2_h, s2_l, 128)))
    return wa, wtb, wts


def _build_x_ops(data):
    """Global col matrices for ops (xe, xo, xs0, xs1, xss) x (hi, lo):
    element k at col PADL + k.  Returns (list[list[np]], PADL)."""
    import ml_dtypes
    NB = N // B
    blocks = data.reshape(NB, B)
    KH = NCORES * HB_PER_CORE
    xe = np.zeros((KH, B), np.float32)
    xo = np.zeros((KH, B), np.float32)
    xe[:NB // 2] = blocks[0::2]
    xo[:NB // 2] = blocks[1::2]
    xs = xe + xo
    xo0 = xo[0::2]
    xo1 = xo[1::2]
    xos = xo0 + xo1
    xs0 = xs[0::2]
    xs1 = xs[1::2]
    xss = xs0 + xs1
    PADL = 132
    out = []
    for op in (xe, xo0, xo1, xos, xs0, xs1, xss):
        hi = _q8(op * SX)
        lo = _q8(op * SX - hi.astype(np.float32))
        mats = []
        for strm in (hi, lo):
            g = np.zeros((B, PADL + KH + 8), dtype=ml_dtypes.float8_e4m3)
            g[:, PADL:PADL + len(strm)] = strm.T
            mats.append(g)
        out.append(mats)
    return out, PADL


def _shard(xops, PADL, i):
    """Core i input: [B, 7, 2, 2, XW] fp8."""
    import ml_dtypes
    K0h = i * HB_PER_CORE
    K0q = i * QB_PER_CORE
    x = np.empty((B, 7, 2, 2, XW), dtype=ml_dtypes.float8_e4m3)
    # (op index, base element index, extra left shift)
    specs = ((0, K0h, 128), (1, K0q, 129), (2, K0q, 130), (3, K0q, 129),
             (4, K0q, 128), (5, K0q, 129), (6, K0q, 128))
    for op, k0, shift in specs:
        base = PADL + k0 - shift
        for st in range(2):
            g = xops[op][st]
            x[:, op, st, 0, :] = g[:, base:base + XW]
            x[:, op, st, 1, :] = g[:, base - 1:base - 1 + XW]
    return x


def kernel(data, rir):
    global _NC_CACHE
    from concourse.bass_utils import run_bass_kernel_spmd

    data = np.asarray(data, dtype=np.float32).reshape(-1)
    h = np.asarray(rir, dtype=np.float32).reshape(-1)

    if _NC_CACHE is None:
        _NC_CACHE = _build_nc()
    nc = _NC_CACHE

    wa, wtb, wts = _build_weights(h)
    xops, PADL = _build_x_ops(data)
    in_maps = [{"x": _shard(xops, PADL, i), "wa": wa, "wtb": wtb,
                "wts": wts} for i in range(NCORES)]
    res = run_bass_kernel_spmd(nc, in_maps, core_ids=list(range(NCORES)))

    y = np.empty(NCORES * BLK_PER_CORE * B, np.float32)
    span = BLK_PER_CORE * B
    for i in range(NCORES):
        y[i * span:(i + 1) * span] = \
            res.results[i]["y"].reshape(B, BLK_PER_CORE).T.reshape(-1)
    return y[:NOUT]
